# revision 15
# baseline (speedup 1.0000x reference)
"""Trainium2 Bass kernel for a dense pre-LN transformer block.

Sharding (8 NeuronCores):
  - 2 groups of 4 cores; group g handles batch g.
  - Within a group, attention is head-parallel: core owns 4 of 16 heads over
    the full 2048-token batch.
  - After attention, a per-head 8-way AllToAll re-shards the attention output
    from head-parallel to token-parallel (each core ends with all 1024
    o-features for its own 512 tokens).  Shards destined to the other batch
    group carry duplicate data and are neutralized by zero rows in a per-core
    zero-padded Wproj (the program is rank-invariant; only input data differs).
  - proj, LN2 and the MLP are token-parallel: core computes its own 512 rows.

Everything is fp32.  The softmax is computed without max-subtraction (scores
are O(1) here), with the denominator produced by a ones-column appended to V
inside the same PE accumulation.
"""

import os
import sys

if "/opt/trn_rl_repo" not in sys.path:
    sys.path.insert(0, "/opt/trn_rl_repo")

import numpy as np

import concourse.bass as bass
import concourse.mybir as mybir
from concourse import bacc, tile
from concourse import bass_utils

B, S, D, H = 2, 2048, 1024, 16
HS = D // H            # 64
EPS = 1e-5
N_CORES = 8
GROUP = 4              # cores per batch group
HPC = 4                # heads per core
TOK = S // GROUP       # own tokens per core (512)

F32 = mybir.dt.float32
AF = mybir.ActivationFunctionType
ALU = mybir.AluOpType

_CACHE = {}


_PHASES = ["A", "B", "C0", "C", "D", "E", "F"]


def _build(phase="F"):
    nc = bacc.Bacc("TRN2", target_bir_lowering=False, debug=False,
                   enable_asserts=True, num_devices=N_CORES)

    # ---- per-core external inputs (data differs per core, program is SPMD) ----
    xb = nc.dram_tensor("xb", [S, D], F32, kind="ExternalInput").ap()            # batch tokens
    wq = nc.dram_tensor("wq", [8, 2, 128, 128], F32, kind="ExternalInput").ap()  # [dt, pt, d, col] (pre-scaled)
    wk = nc.dram_tensor("wk", [8, 2, 128, 128], F32, kind="ExternalInput").ap()
    wv = nc.dram_tensor("wv", [8, 128, 256], F32, kind="ExternalInput").ap()     # [dt, d, e]
    wpd = nc.dram_tensor("wpd", [16, 128, D], F32, kind="ExternalInput").ap()    # zero-padded Wproj
    w1 = nc.dram_tensor("w1", [32, 8, 128, 128], F32, kind="ExternalInput").ap() # [fc, dt, d, f]
    w2 = nc.dram_tensor("w2", [32, 2, 128, 512], F32, kind="ExternalInput").ap() # [ft, dc, f, d]
    xpb = nc.dram_tensor("xpb", [4, 128, D], F32, kind="ExternalInput").ap()     # x_own + bproj
    g1 = nc.dram_tensor("g1", [128, 8], F32, kind="ExternalInput").ap()
    b1n = nc.dram_tensor("b1n", [128, 8], F32, kind="ExternalInput").ap()
    g2 = nc.dram_tensor("g2", [128, 8], F32, kind="ExternalInput").ap()
    b2n = nc.dram_tensor("b2n", [128, 8], F32, kind="ExternalInput").ap()
    b1t = nc.dram_tensor("b1t", [128, 32], F32, kind="ExternalInput").ap()
    b2b = nc.dram_tensor("b2b", [128, D], F32, kind="ExternalInput").ap()
    ident = nc.dram_tensor("ident", [128, 128], F32, kind="ExternalInput").ap()

    y = nc.dram_tensor("y", [TOK, D], F32, kind="ExternalOutput").ap()

    dbg = {}
    lvl = _PHASES.index(phase)
    if lvl < _PHASES.index("F"):
        shapes = {
            "A": {"h1T": [128, 8 * S]},
            "B": {"QT": [128, 2 * S], "KT": [128, 2 * S], "Vt": [128, 16 * 4 * 65]},
            "C0": {f"dbin{h}": [512, 512] for h in range(HPC)},
            "C": {f"dbout{h}": [512, 512] for h in range(HPC)},
            "D": {"x2": [128, 4 * D]},
            "E": {"h2T": [128, 8 * TOK]},
        }[phase]
        for k, shp in shapes.items():
            dbg[k] = nc.dram_tensor(f"dbg_{k}", shp, F32,
                                    kind="ExternalOutput").ap()

    with tile.TileContext(nc) as tc:
        _emit(nc, tc, xb, wq, wk, wv, wpd, w1, w2, xpb,
              g1, b1n, g2, b2n, b1t, b2b, ident, y, phase=phase, dbg=dbg)
    nc.compile()
    return nc


def _ln_tile(nc, lns, src_ap, tag):
    """LayerNorm stats for one [128, 1024] tile -> (mean, rstd) [128,1] each.

    rstd gets one Newton refinement (ACT Sqrt has a loose ULP budget).
    """
    stats = lns.tile([128, 2, 6], F32, tag=f"{tag}stats")
    nc.vector.bn_stats(stats[:, 0, :], src_ap[:, 0:512])
    nc.vector.bn_stats(stats[:, 1, :], src_ap[:, 512:1024])
    mv = lns.tile([128, 2], F32, tag=f"{tag}mv")
    nc.vector.bn_aggr(mv[:], stats[:])
    ve = lns.tile([128, 1], F32, tag=f"{tag}ve")
    nc.vector.tensor_scalar_add(ve[:], mv[:, 1:2], EPS)
    std = lns.tile([128, 1], F32, tag=f"{tag}std")
    nc.scalar.activation(std[:], ve[:], AF.Sqrt)
    r0 = lns.tile([128, 1], F32, tag=f"{tag}r0")
    nc.vector.reciprocal(r0[:], std[:])
    t1 = lns.tile([128, 1], F32, tag=f"{tag}t1")
    nc.vector.tensor_mul(t1[:], r0[:], r0[:])
    nc.vector.tensor_mul(t1[:], t1[:], ve[:])
    nc.vector.tensor_scalar(t1[:], t1[:], -0.5, 1.5, ALU.mult, ALU.add)
    r1 = lns.tile([128, 1], F32, tag=f"{tag}r1")
    nc.vector.tensor_mul(r1[:], t1[:], r0[:])
    return mv[:, 0:1], r1


def _emit(nc, tc, xb, wq, wk, wv, wpd, w1, w2, xpb,
          g1, b1n, g2, b2n, b1t, b2b, ident, y, phase="F", dbg=None):
    lvl = _PHASES.index(phase)

    def onward(p):
        return lvl >= _PHASES.index(p)

    with tc.tile_pool(name="const", bufs=1) as const, \
         tc.tile_pool(name="ps2", bufs=2, space="PSUM") as ps2, \
         tc.tile_pool(name="ps_av", bufs=2, space="PSUM") as ps_av, \
         tc.tile_pool(name="ps_bc", bufs=1, space="PSUM") as ps_bc, \
         tc.tile_pool(name="dram", bufs=1, space="DRAM") as dram:

        # ---------- constants ----------
        idt = const.tile([128, 128], F32)
        nc.sync.dma_start(idt[:], ident[:])
        g1_s = const.tile([128, 8], F32, tag="g1"); nc.sync.dma_start(g1_s[:], g1[:])
        b1n_s = const.tile([128, 8], F32, tag="b1n"); nc.sync.dma_start(b1n_s[:], b1n[:])
        g2_s = const.tile([128, 8], F32, tag="g2"); nc.sync.dma_start(g2_s[:], g2[:])
        b2n_s = const.tile([128, 8], F32, tag="b2n"); nc.sync.dma_start(b2n_s[:], b2n[:])
        b1t_s = const.tile([128, 32], F32, tag="b1t"); nc.sync.dma_start(b1t_s[:], b1t[:])
        b2b_s = const.tile([128, D], F32, tag="b2b"); nc.sync.dma_start(b2b_s[:], b2b[:])
        xpb_s = const.tile([128, 4, D], F32, tag="xpb")
        nc.sync.dma_start(xpb_s[:], xpb.rearrange("st p d -> p st d"))
        ones64 = const.tile([1, 64], F32, tag="ones64")
        nc.vector.memset(ones64[:], 1.0)

        bins = [dram.tile([512, 512], F32, tag=f"bin{h}", name=f"bin{h}")
                for h in range(HPC)]
        bouts = [dram.tile([512, 512], F32, tag=f"bout{h}", name=f"bout{h}")
                 for h in range(HPC)]

        with tc.tile_pool(name="qkv", bufs=1) as qkv_pool:
            QT = qkv_pool.tile([128, 2, S], F32, tag="QT")
            KT = qkv_pool.tile([128, 2, S], F32, tag="KT")
            Vt = qkv_pool.tile([128, 16, 4, 65], F32, tag="Vt")
            nc.vector.memset(Vt[:, :, :, 64:65], 1.0)

            # =====================================================
            # Phase A: LN1 over the full batch + transpose -> h1T
            # h1T layout: [128 (d inner), 8 (d outer), 2048 (tokens)]
            # =====================================================
            with tc.tile_pool(name="h1", bufs=1) as h1_pool:
                h1T = h1_pool.tile([128, 8, S], F32, tag="h1T")
                with tc.tile_pool(name="ln1", bufs=3) as lnp, \
                     tc.tile_pool(name="ln1s", bufs=4) as lns:
                    for st in range(16):
                        xt = lnp.tile([128, D], F32, tag="xt")
                        nc.sync.dma_start(xt[:], xb[128 * st:128 * (st + 1), :])
                        mean, rstd = _ln_tile(nc, lns, xt[:], "a")
                        pn = lnp.tile([128, D], F32, tag="pn")
                        nc.vector.tensor_scalar(pn[:], xt[:], mean, rstd[:],
                                                ALU.subtract, ALU.mult)
                        pst = ps2.tile([128, 1024], F32, tag="ps2")
                        for dt in range(8):
                            nc.tensor.transpose(pst[:, 128 * dt:128 * (dt + 1)],
                                                pn[:, 128 * dt:128 * (dt + 1)],
                                                idt[:])
                        nc.vector.tensor_copy(
                            h1T[:, :, 128 * st:128 * (st + 1)],
                            pst[:].rearrange("p (dt t) -> p dt t", dt=8))
                for dt in range(8):
                    nc.vector.tensor_scalar(h1T[:, dt, :], h1T[:, dt, :],
                                            g1_s[:, dt:dt + 1], b1n_s[:, dt:dt + 1],
                                            ALU.mult, ALU.add)
                if phase == "A":
                    nc.sync.dma_start(
                        dbg["h1T"].rearrange("p (a b) -> p a b", a=8), h1T[:])

                # =================================================
                # Phase B: QKV for own 4 heads over the full batch
                # QT/KT: [128, 2, 2048] (tile pt = heads {2pt, 2pt+1})
                # V: [128, 16, 4, 65] ([t_in, t_out, head, e|ones])
                # =================================================
                with tc.tile_pool(name="wqk", bufs=4) as wqk:
                    if not onward("B"):
                        return
                    for wten, dst in ((wq, QT), (wk, KT)):
                        for pt in range(2):
                            for tc4 in range(4):
                                acc = ps2.tile([128, 1024], F32, tag="ps2")
                                for dt in range(8):
                                    wt = wqk.tile([128, 128], F32, tag="wt")
                                    nc.sync.dma_start(wt[:], wten[dt, pt])
                                    nc.tensor.matmul(
                                        acc[:, 0:512], wt[:],
                                        h1T[:, dt, 512 * tc4:512 * (tc4 + 1)],
                                        start=(dt == 0), stop=(dt == 7))
                                nc.vector.tensor_copy(
                                    dst[:, pt, 512 * tc4:512 * (tc4 + 1)],
                                    acc[:, 0:512])
                    for tc16 in range(16):
                        acc = ps2.tile([128, 1024], F32, tag="ps2")
                        for dt in range(8):
                            wt = wqk.tile([128, 256], F32, tag="wv")
                            nc.sync.dma_start(wt[:], wv[dt])
                            nc.tensor.matmul(
                                acc[:, 0:256],
                                h1T[:, dt, 128 * tc16:128 * (tc16 + 1)],
                                wt[:], start=(dt == 0), stop=(dt == 7))
                        nc.vector.tensor_copy(
                            Vt[:, tc16, :, 0:64],
                            acc[:, 0:256].rearrange("p (h e) -> p h e", h=4))

            if phase == "B":
                nc.sync.dma_start(
                    dbg["QT"].rearrange("p (a b) -> p a b", a=2), QT[:])
                nc.sync.dma_start(
                    dbg["KT"].rearrange("p (a b) -> p a b", a=2), KT[:])
                nc.sync.dma_start(
                    dbg["Vt"], Vt[:].rearrange("p a b c -> p (a b c)"))
                return

            # =====================================================
            # Phase C: scores + softmax + AV per head pair; A2A
            # =====================================================
            with tc.tile_pool(name="et", bufs=4) as etp, \
                 tc.tile_pool(name="att", bufs=3) as att:
                for hp in range(2):
                    for sb in range(4):
                        pav = [ps_av.tile([65, 512], F32, tag="pav",
                                          name=f"pav{hp}_{sb}_{i}")
                               for i in range(2)]
                        for tcc in range(16):
                            psc = ps2.tile([128, 1024], F32, tag="ps2")
                            for hq in range(2):
                                nc.tensor.matmul(
                                    psc[:, 512 * hq:512 * (hq + 1)],
                                    KT[64 * hq:64 * (hq + 1), hp,
                                       128 * tcc:128 * (tcc + 1)],
                                    QT[64 * hq:64 * (hq + 1), hp,
                                       512 * sb:512 * (sb + 1)],
                                    start=True, stop=True)
                            et = etp.tile([128, 1024], F32, tag="et")
                            nc.scalar.activation(et[:], psc[:], AF.Exp)
                            for hq in range(2):
                                nc.tensor.matmul(
                                    pav[hq][:],
                                    Vt[:, tcc, 2 * hp + hq, :],
                                    et[:, 512 * hq:512 * (hq + 1)],
                                    start=(tcc == 0), stop=(tcc == 15))
                        for hq in range(2):
                            h = 2 * hp + hq
                            rec = att.tile([1, 512], F32, tag="rec")
                            nc.vector.reciprocal(rec[:], pav[hq][64:65, :])
                            pbc = ps_bc.tile([64, 512], F32, tag="pbc")
                            nc.tensor.matmul(pbc[:], ones64[:], rec[:],
                                             start=True, stop=True)
                            rb = att.tile([64, 512], F32, tag="rb")
                            nc.vector.tensor_copy(rb[:], pbc[:])
                            ot = att.tile([64, 512], F32, tag="ot")
                            nc.vector.tensor_mul(ot[:], pav[hq][0:64, :], rb[:])
                            # both group slots get the data (finite everywhere)
                            nc.sync.dma_start(
                                bins[h][64 * sb:64 * (sb + 1), :], ot[:])
                            nc.sync.dma_start(
                                bins[h][64 * (sb + 4):64 * (sb + 5), :], ot[:])
                if phase == "C0":
                    for h in range(HPC):
                        nc.sync.dma_start(dbg[f"dbin{h}"], bins[h][:])
                    return
                for h in range(HPC):
                    nc.gpsimd.collective_compute(
                        "AllToAll", ALU.bypass,
                        replica_groups=[list(range(N_CORES))],
                        ins=[bins[h].opt()], outs=[bouts[h].opt()])
                if phase == "C":
                    for h in range(HPC):
                        nc.sync.dma_start(dbg[f"dbout{h}"], bouts[h][:])
                    return

        # =========================================================
        # Phase D: proj (own 512 tokens) + residual -> x2
        # =========================================================
        with tc.tile_pool(name="x2p", bufs=1) as x2_pool:
            x2 = x2_pool.tile([128, 4, D], F32, tag="x2")

            with tc.tile_pool(name="proj", bufs=1) as prj:
                pin = prj.tile([128, 16, 512], F32, tag="pin")
                for h in range(HPC):
                    for q in range(4):
                        nc.sync.dma_start(pin[:, 4 * h + q, :],
                                          bouts[h][128 * q:128 * (q + 1), :])
                wpd_s = prj.tile([128, 16, D], F32, tag="wpd")
                nc.sync.dma_start(wpd_s[:], wpd.rearrange("et p d -> p et d"))
                for sc in range(4):
                    for dc in range(2):
                        acc = ps2.tile([128, 1024], F32, tag="ps2")
                        for et in range(16):
                            nc.tensor.matmul(
                                acc[:, 0:512],
                                pin[:, et, 128 * sc:128 * (sc + 1)],
                                wpd_s[:, et, 512 * dc:512 * (dc + 1)],
                                start=(et == 0), stop=(et == 15))
                        nc.vector.tensor_add(
                            x2[:, sc, 512 * dc:512 * (dc + 1)],
                            acc[:, 0:512],
                            xpb_s[:, sc, 512 * dc:512 * (dc + 1)])

            if phase == "D":
                nc.sync.dma_start(
                    dbg["x2"].rearrange("p (a b) -> p a b", a=4), x2[:])
                return

            # =====================================================
            # Phase E/F: LN2 -> h2T ; MLP ; output
            # =====================================================
            with tc.tile_pool(name="mt", bufs=1) as mt_pool:
                mT = mt_pool.tile([128, 32, TOK], F32, tag="mT")

                with tc.tile_pool(name="h2", bufs=1) as h2_pool:
                    h2T = h2_pool.tile([128, 8, TOK], F32, tag="h2T")
                    with tc.tile_pool(name="ln2", bufs=2) as lnp2, \
                         tc.tile_pool(name="ln2s", bufs=4) as lns2:
                        for st in range(4):
                            mean, rstd = _ln_tile(nc, lns2, x2[:, st, :], "b")
                            pn = lnp2.tile([128, D], F32, tag="pn2")
                            nc.vector.tensor_scalar(pn[:], x2[:, st, :],
                                                    mean, rstd[:],
                                                    ALU.subtract, ALU.mult)
                            pst = ps2.tile([128, 1024], F32, tag="ps2")
                            for dt in range(8):
                                nc.tensor.transpose(
                                    pst[:, 128 * dt:128 * (dt + 1)],
                                    pn[:, 128 * dt:128 * (dt + 1)], idt[:])
                            nc.vector.tensor_copy(
                                h2T[:, :, 128 * st:128 * (st + 1)],
                                pst[:].rearrange("p (dt t) -> p dt t", dt=8))
                    for dt in range(8):
                        nc.vector.tensor_scalar(h2T[:, dt, :], h2T[:, dt, :],
                                                g2_s[:, dt:dt + 1],
                                                b2n_s[:, dt:dt + 1],
                                                ALU.mult, ALU.add)
                    if phase == "E":
                        nc.sync.dma_start(
                            dbg["h2T"].rearrange("p (a b) -> p a b", a=8),
                            h2T[:])
                        return

                    # MLP up-projection: mT = relu(W1^T h2 + b1)
                    with tc.tile_pool(name="w1s", bufs=6) as w1s:
                        for fc in range(32):
                            acc = ps2.tile([128, 1024], F32, tag="ps2")
                            for dt in range(8):
                                wt = w1s.tile([128, 128], F32, tag="w1t")
                                nc.sync.dma_start(wt[:], w1[fc, dt])
                                nc.tensor.matmul(acc[:, 0:512], wt[:],
                                                 h2T[:, dt, :],
                                                 start=(dt == 0), stop=(dt == 7))
                            nc.scalar.activation(mT[:, fc, :], acc[:, 0:512],
                                                 AF.Relu,
                                                 bias=b1t_s[:, fc:fc + 1])

                # MLP down-projection + residual + b2 -> y
                with tc.tile_pool(name="w2s", bufs=4) as w2s, \
                     tc.tile_pool(name="yout", bufs=3) as yp:
                    for sc in range(4):
                        for dc in range(2):
                            acc = ps2.tile([128, 1024], F32, tag="ps2")
                            for ft in range(32):
                                wt = w2s.tile([128, 512], F32, tag="w2t")
                                nc.sync.dma_start(wt[:], w2[ft, dc])
                                nc.tensor.matmul(
                                    acc[:, 0:512],
                                    mT[:, ft, 128 * sc:128 * (sc + 1)],
                                    wt[:], start=(ft == 0), stop=(ft == 31))
                            yt = yp.tile([128, 512], F32, tag="yt")
                            nc.vector.tensor_add(
                                yt[:], acc[:, 0:512],
                                x2[:, sc, 512 * dc:512 * (dc + 1)])
                            nc.vector.tensor_add(
                                yt[:], yt[:],
                                b2b_s[:, 512 * dc:512 * (dc + 1)])
                            nc.sync.dma_start(
                                y[128 * sc:128 * (sc + 1),
                                  512 * dc:512 * (dc + 1)],
                                yt[:])


def _get_nc(phase="F"):
    key = f"nc_{phase}"
    if key not in _CACHE:
        _CACHE[key] = _build(phase)
    return _CACHE[key]


def _prep_in_maps(x, Wq, Wk, Wv, Wproj, bproj, ln1_g, ln1_b, ln2_g, ln2_b,
                  W1, b1, W2, b2):
    f = np.float32
    x = np.asarray(x, f)
    Wq = np.asarray(Wq, f); Wk = np.asarray(Wk, f); Wv = np.asarray(Wv, f)
    Wproj = np.asarray(Wproj, f); bproj = np.asarray(bproj, f)
    ln1_g = np.asarray(ln1_g, f); ln1_b = np.asarray(ln1_b, f)
    ln2_g = np.asarray(ln2_g, f); ln2_b = np.asarray(ln2_b, f)
    W1 = np.asarray(W1, f); b1 = np.asarray(b1, f)
    W2 = np.asarray(W2, f); b2 = np.asarray(b2, f)

    scale = f(HS) ** f(-0.5)
    ident = np.eye(128, dtype=f)
    g1v = np.ascontiguousarray(ln1_g.reshape(8, 128).T)
    b1v = np.ascontiguousarray(ln1_b.reshape(8, 128).T)
    g2v = np.ascontiguousarray(ln2_g.reshape(8, 128).T)
    b2v = np.ascontiguousarray(ln2_b.reshape(8, 128).T)
    b1tv = np.ascontiguousarray(b1.reshape(32, 128).T)
    b2bv = np.ascontiguousarray(np.tile(b2[None, :], (128, 1)))
    w1v = np.ascontiguousarray(
        W1.reshape(8, 128, 32, 128).transpose(2, 0, 1, 3))
    w2v = np.ascontiguousarray(
        W2.reshape(32, 128, 2, 512).transpose(0, 2, 1, 3))

    in_maps = []
    for c in range(N_CORES):
        g, r = divmod(c, GROUP)
        heads = [4 * r + h for h in range(HPC)]
        wq_c = np.concatenate([Wq[h] for h in heads], axis=1) * scale  # [D, 256]
        wk_c = np.concatenate([Wk[h] for h in heads], axis=1)
        wv_c = np.concatenate([Wv[h] for h in heads], axis=1)
        wq_r = np.ascontiguousarray(
            wq_c.reshape(8, 128, 2, 128).transpose(0, 2, 1, 3))
        wk_r = np.ascontiguousarray(
            wk_c.reshape(8, 128, 2, 128).transpose(0, 2, 1, 3))
        wv_r = np.ascontiguousarray(wv_c.reshape(8, 128, 256))
        # zero-padded proj: e-tile (4*h_slot + i//2), rows 64*(i%2)
        wpd_v = np.zeros((16, 128, D), dtype=f)
        for h_slot in range(HPC):
            for i in range(N_CORES):
                if i // GROUP == g:
                    gh = 4 * (i % GROUP) + h_slot
                    blk = 64 * (i % 2)
                    wpd_v[4 * h_slot + i // 2, blk:blk + 64, :] = \
                        Wproj[64 * gh:64 * (gh + 1), :]
        xrows = x[g, TOK * r:TOK * (r + 1), :] + bproj
        in_maps.append({
            "xb": np.ascontiguousarray(x[g]),
            "wq": wq_r, "wk": wk_r, "wv": wv_r,
            "wpd": wpd_v, "w1": w1v, "w2": w2v,
            "xpb": np.ascontiguousarray(xrows.reshape(4, 128, D)),
            "g1": g1v, "b1n": b1v, "g2": g2v, "b2n": b2v,
            "b1t": b1tv, "b2b": b2bv, "ident": ident,
        })
    return in_maps


def run(inputs, trace=False, phase="F"):
    nc = _get_nc(phase)
    in_maps = _prep_in_maps(**inputs)
    res = bass_utils.run_bass_kernel_spmd(
        nc, in_maps, core_ids=list(range(N_CORES)), trace=trace)
    if phase != "F":
        return res.results, res.exec_time_ns
    out = np.empty((B, S, D), dtype=np.float32)
    for c in range(N_CORES):
        g, r = divmod(c, GROUP)
        out[g, TOK * r:TOK * (r + 1), :] = res.results[c]["y"]
    return out, res.exec_time_ns


def kernel(**inputs):
    out, _ = run(inputs)
    return out


# revision 24
# speedup vs baseline: 2.3231x; 2.3231x over previous
"""Trainium2 Bass kernel for a dense pre-LN transformer block.

Sharding (8 NeuronCores):
  - 2 groups of 4 cores; group g handles batch g.
  - Within a group, attention is head-parallel: core owns 4 of 16 heads over
    the full 2048-token batch.
  - After attention, one 8-way AllToAll re-shards the attention output from
    head-parallel to token-parallel (each core ends with all 1024 o-features
    for its own 512 tokens).  Shards destined to the other batch group carry
    duplicate data and are neutralized by zero rows in a per-core zero-padded
    Wproj (the program is rank-invariant; only input data differs).
  - proj, LN2 and the MLP are token-parallel: core computes its own 512 rows.

Matmul inputs are float32r (tf32); accumulation, layernorm, softmax
normalization and the residual stream stay fp32.  The softmax needs no
max-subtraction (scores are O(1)); the denominator comes from a ones-column
appended to V inside the same PE accumulation.  LayerNorm gains/biases are
folded into the adjacent weights on the host.
"""

import os
import sys

if "/opt/trn_rl_repo" not in sys.path:
    sys.path.insert(0, "/opt/trn_rl_repo")

import numpy as np

import concourse.bass as bass
import concourse.mybir as mybir
from concourse import bacc, tile
from concourse import bass_utils

B, S, D, H = 2, 2048, 1024, 16
HS = D // H            # 64
EPS = 1e-5
N_CORES = 8
GROUP = 4              # cores per batch group
HPC = 4                # heads per core
TOK = S // GROUP       # own tokens per core (512)

F32 = mybir.dt.float32
F32R = mybir.dt.float32r
RDT = F32 if os.environ.get("KF32") else F32R
AF = mybir.ActivationFunctionType
ALU = mybir.AluOpType

_CACHE = {}

_PHASES = ["A", "B", "C0", "C", "D", "E", "F"]


def _build(phase="F"):
    nc = bacc.Bacc("TRN2", target_bir_lowering=False, debug=False,
                   enable_asserts=True, num_devices=N_CORES)

    tens = {}
    tens["xb"] = nc.dram_tensor("xb", [S, D], F32, kind="ExternalInput").ap()
    # QKV weights with LN1 gain folded; wq also carries the 1/sqrt(HS) scale.
    tens["wq"] = nc.dram_tensor("wq", [128, 8, 2, 128], RDT, kind="ExternalInput").ap()
    tens["wk"] = nc.dram_tensor("wk", [128, 8, 2, 128], RDT, kind="ExternalInput").ap()
    tens["wv"] = nc.dram_tensor("wv", [128, 8, 256], RDT, kind="ExternalInput").ap()
    tens["bqk"] = nc.dram_tensor("bqk", [128, 2, 2], F32, kind="ExternalInput").ap()
    tens["bv"] = nc.dram_tensor("bv", [1, 256], RDT, kind="ExternalInput").ap()
    tens["wpd"] = nc.dram_tensor("wpd", [16, 128, D], RDT, kind="ExternalInput").ap()
    tens["w1"] = nc.dram_tensor("w1", [32, 128, 8, 128], RDT, kind="ExternalInput").ap()
    tens["w2"] = nc.dram_tensor("w2", [2, 32, 128, 512], RDT, kind="ExternalInput").ap()
    tens["xpb"] = nc.dram_tensor("xpb", [4, 128, D], F32, kind="ExternalInput").ap()
    tens["b1t"] = nc.dram_tensor("b1t", [128, 32], F32, kind="ExternalInput").ap()
    tens["b2b"] = nc.dram_tensor("b2b", [128, D], F32, kind="ExternalInput").ap()
    tens["ident"] = nc.dram_tensor("ident", [128, 128], F32, kind="ExternalInput").ap()
    tens["onesc"] = nc.dram_tensor("onesc", [128, 128], RDT, kind="ExternalInput").ap()
    tens["y"] = nc.dram_tensor("y", [TOK, D], F32, kind="ExternalOutput").ap()

    dbg = {}
    if phase != "F":
        shapes = {
            "A": {"h1T": [128, 8 * S]},
            "B": {"QT": [128, 2 * S], "KT": [128, 2 * S], "Vt": [128, 16 * 4 * 65]},
            "C0": {"dbin": [2048, 512]},
            "C": {"dbout": [2048, 512]},
            "D": {"x2": [128, 4 * D]},
            "E": {"h2T": [128, 8 * TOK]},
        }[phase]
        for k, shp in shapes.items():
            dbg[k] = nc.dram_tensor(f"dbg_{k}", shp, F32,
                                    kind="ExternalOutput").ap()

    with tile.TileContext(nc) as tc:
        with nc.allow_low_precision(reason="tf32 matmul inputs by design"):
            _emit(nc, tc, tens, phase, dbg)
    nc.compile()
    return nc


def _ln_tile(nc, lns, src_ap, tag):
    """LayerNorm stats for one [128, 1024] tile -> (mean, rstd) [128,1].

    rstd gets one Newton refinement (ACT Sqrt has a loose ULP budget).
    """
    stats = lns.tile([128, 2, 6], F32, tag=f"{tag}stats")
    nc.vector.bn_stats(stats[:, 0, :], src_ap[:, 0:512])
    nc.vector.bn_stats(stats[:, 1, :], src_ap[:, 512:1024])
    mv = lns.tile([128, 2], F32, tag=f"{tag}mv")
    nc.vector.bn_aggr(mv[:], stats[:])
    ve = lns.tile([128, 1], F32, tag=f"{tag}ve")
    nc.vector.tensor_scalar_add(ve[:], mv[:, 1:2], EPS)
    std = lns.tile([128, 1], F32, tag=f"{tag}std")
    nc.scalar.activation(std[:], ve[:], AF.Sqrt)
    r0 = lns.tile([128, 1], F32, tag=f"{tag}r0")
    nc.vector.reciprocal(r0[:], std[:])
    t1 = lns.tile([128, 1], F32, tag=f"{tag}t1")
    nc.vector.tensor_mul(t1[:], r0[:], r0[:])
    nc.vector.tensor_mul(t1[:], t1[:], ve[:])
    nc.vector.tensor_scalar(t1[:], t1[:], -0.5, 1.5, ALU.mult, ALU.add)
    r1 = lns.tile([128, 1], F32, tag=f"{tag}r1")
    nc.vector.tensor_mul(r1[:], t1[:], r0[:])
    return mv[:, 0:1], r1


def _ln_transpose(nc, tc, ps2, lnp, lns, src_ap, dstT, st, idt, tag):
    """One [128,1024] tile: LN stats + prenorm + PE transpose into dstT."""
    mean, rstd = _ln_tile(nc, lns, src_ap, tag)
    pn = lnp.tile([128, D], F32, tag=f"{tag}pn")
    nc.vector.tensor_scalar(pn[:], src_ap, mean, rstd[:],
                            ALU.subtract, ALU.mult)
    pst = ps2.tile([128, 1024], F32, tag="ps2")
    for dt in range(8):
        nc.tensor.transpose(pst[:, 128 * dt:128 * (dt + 1)],
                            pn[:, 128 * dt:128 * (dt + 1)], idt[:])
    nc.vector.tensor_copy(
        dstT[:, :, 128 * st:128 * (st + 1)],
        pst[:].rearrange("p (dt t) -> p dt t", dt=8))


def _emit(nc, tc, tens, phase, dbg):
    xb, wpd, w1, w2, xpb, y = (tens["xb"], tens["wpd"], tens["w1"],
                               tens["w2"], tens["xpb"], tens["y"])

    with tc.tile_pool(name="const", bufs=1) as const, \
         tc.tile_pool(name="ps2", bufs=2, space="PSUM") as ps2, \
         tc.tile_pool(name="ps_av", bufs=3, space="PSUM") as ps_av, \
         tc.tile_pool(name="ps_bc", bufs=1, space="PSUM") as ps_bc, \
         tc.tile_pool(name="dram", bufs=1, space="DRAM") as dram, \
         tc.tile_pool(name="w1s", bufs=4) as w1s, \
         tc.tile_pool(name="w2s", bufs=4) as w2s:

        # ---------- constants (sync queue) ----------
        idt = const.tile([128, 128], F32)
        nc.sync.dma_start(idt[:], tens["ident"][:])
        b1t_s = const.tile([128, 32], F32, tag="b1t")
        nc.sync.dma_start(b1t_s[:], tens["b1t"][:])
        b2b_s = const.tile([128, D], F32, tag="b2b")
        nc.sync.dma_start(b2b_s[:], tens["b2b"][:])
        onesc_s = const.tile([128, 128], RDT, tag="onesc")
        nc.sync.dma_start(onesc_s[:], tens["onesc"][:])
        ones64 = onesc_s[0:1, 0:64]
        onestok = onesc_s[0:1, :]
        bqk_s = const.tile([128, 2, 2], F32, tag="bqk")
        nc.sync.dma_start(bqk_s[:], tens["bqk"][:])
        bv_s = const.tile([1, 256], RDT, tag="bv")
        nc.sync.dma_start(bv_s[:], tens["bv"][:])
        wq_s = const.tile([128, 8, 2, 128], RDT, tag="wq")
        nc.sync.dma_start(wq_s[:], tens["wq"][:])
        wk_s = const.tile([128, 8, 2, 128], RDT, tag="wk")
        nc.sync.dma_start(wk_s[:], tens["wk"][:])
        wv_s = const.tile([128, 8, 256], RDT, tag="wv")
        nc.sync.dma_start(wv_s[:], tens["wv"][:])

        bin_all = dram.tile([2048, 512], RDT, tag="bin_all", name="bin_all")
        bout_all = dram.tile([2048, 512], RDT, tag="bout_all", name="bout_all")

        # ====================== attention half ======================
        with tc.tile_pool(name="qkv", bufs=1) as qkv_pool:
            QT = qkv_pool.tile([128, 2, S], RDT, tag="QT")
            KT = qkv_pool.tile([128, 2, S], RDT, tag="KT")
            Vt = qkv_pool.tile([128, 16, 4, 65], RDT, tag="Vt")
            nc.sync.dma_start(
                Vt[:, :, :, 64],
                tens["onesc"][:, 0:64].rearrange("p (a b) -> p a b", a=16))

            # Phases A+B interleaved: LN1 -> h1T; QKV per 512-token chunk.
            # h1T: [128 (d in), 8 (d out), 2048]; QT/KT: [128, 2, 2048]
            # (partition tile pt = heads {2pt, 2pt+1});
            # V: [128, 16, 4, 65] ([t_in, t_out, head, e|ones])
            with tc.tile_pool(name="h1", bufs=1) as h1_pool:
                h1T = h1_pool.tile([128, 8, S], RDT, tag="h1T")
                with tc.tile_pool(name="ln1", bufs=3) as lnp, \
                     tc.tile_pool(name="ln1s", bufs=4) as lns:
                    for tc4 in range(4):
                        for st in range(4 * tc4, 4 * tc4 + 4):
                            xt = lnp.tile([128, D], F32, tag="xt")
                            nc.sync.dma_start(
                                xt[:], xb[128 * st:128 * (st + 1), :])
                            _ln_transpose(nc, tc, ps2, lnp, lns, xt[:],
                                          h1T, st, idt, "a")
                        if phase == "A" and tc4 == 3:
                            nc.sync.dma_start(
                                dbg["h1T"].rearrange("p (a b) -> p a b", a=8),
                                h1T[:])
                            return
                        # ---- QKV for this 512-token chunk ----
                        tsl = slice(512 * tc4, 512 * (tc4 + 1))
                        for wten, dst, col in ((wq_s, QT, 0), (wk_s, KT, 1)):
                            for pt in range(2):
                                acc = ps2.tile([128, 1024], F32, tag="ps2")
                                for dt in range(8):
                                    nc.tensor.matmul(
                                        acc[:, 0:512], wten[:, dt, pt, :],
                                        h1T[:, dt, tsl],
                                        start=(dt == 0), stop=(dt == 7))
                                nc.vector.tensor_scalar_add(
                                    dst[:, pt, tsl], acc[:, 0:512],
                                    bqk_s[:, pt, col:col + 1])
                        for tc16 in range(4 * tc4, 4 * tc4 + 4):
                            acc = ps2.tile([128, 1024], F32, tag="ps2")
                            csl = slice(128 * tc16, 128 * (tc16 + 1))
                            for dt in range(8):
                                nc.tensor.matmul(
                                    acc[:, 0:256], h1T[:, dt, csl],
                                    wv_s[:, dt, :],
                                    start=(dt == 0), stop=False)
                            nc.tensor.matmul(acc[:, 0:256], onestok,
                                             bv_s[:], start=False, stop=True)
                            nc.vector.tensor_copy(
                                Vt[:, tc16, :, 0:64],
                                acc[:, 0:256].rearrange("p (h e) -> p h e",
                                                        h=4))

            if phase == "B":
                nc.sync.dma_start(
                    dbg["QT"].rearrange("p (a b) -> p a b", a=2), QT[:])
                nc.sync.dma_start(
                    dbg["KT"].rearrange("p (a b) -> p a b", a=2), KT[:])
                nc.sync.dma_start(
                    dbg["Vt"], Vt[:].rearrange("p a b c -> p (a b c)"))
                return

            # Phase C: scores + exp + AV per head pair
            with tc.tile_pool(name="et", bufs=6) as etp, \
                 tc.tile_pool(name="att", bufs=3) as att:
                for hp in range(2):
                    for sb in range(4):
                        pav = [ps_av.tile([65, 512], F32, tag="pav",
                                          name=f"pav{hp}_{sb}_{i}")
                               for i in range(2)]
                        for tcc in range(16):
                            psc = ps2.tile([128, 1024], F32, tag="ps2")
                            for hq in range(2):
                                nc.tensor.matmul(
                                    psc[:, 512 * hq:512 * (hq + 1)],
                                    KT[64 * hq:64 * (hq + 1), hp,
                                       128 * tcc:128 * (tcc + 1)],
                                    QT[64 * hq:64 * (hq + 1), hp,
                                       512 * sb:512 * (sb + 1)],
                                    start=True, stop=True)
                            et = etp.tile([128, 1024], RDT, tag="et")
                            nc.scalar.activation(et[:], psc[:], AF.Exp)
                            for hq in range(2):
                                nc.tensor.matmul(
                                    pav[hq][:],
                                    Vt[:, tcc, 2 * hp + hq, :],
                                    et[:, 512 * hq:512 * (hq + 1)],
                                    start=(tcc == 0), stop=(tcc == 15))
                        for hq in range(2):
                            h = 2 * hp + hq
                            rec = att.tile([1, 512], RDT, tag="rec")
                            nc.vector.reciprocal(rec[:], pav[hq][64:65, :])
                            pbc = ps_bc.tile([64, 512], F32, tag="pbc")
                            nc.tensor.matmul(pbc[:], ones64, rec[:],
                                             start=True, stop=True)
                            rb = att.tile([64, 512], F32, tag="rb")
                            nc.vector.tensor_copy(rb[:], pbc[:])
                            ot = att.tile([64, 512], RDT, tag="ot")
                            nc.vector.tensor_mul(ot[:], pav[hq][0:64, :],
                                                 rb[:])
                            # both group slots (keeps wire data finite)
                            nc.gpsimd.dma_start(
                                bin_all[256 * sb + 64 * h:
                                        256 * sb + 64 * (h + 1), :], ot[:])
                            nc.gpsimd.dma_start(
                                bin_all[256 * (sb + 4) + 64 * h:
                                        256 * (sb + 4) + 64 * (h + 1), :],
                                ot[:])
            if phase == "C0":
                nc.sync.dma_start(dbg["dbin"], bin_all[:])
                return
            nc.gpsimd.collective_compute(
                "AllToAll", ALU.bypass,
                replica_groups=[list(range(N_CORES))],
                ins=[bin_all.opt()], outs=[bout_all.opt()])
            if phase == "C":
                nc.sync.dma_start(dbg["dbout"], bout_all[:])
                return

        # ====================== token-local half ======================
        with tc.tile_pool(name="x2p", bufs=1) as x2_pool:
            x2 = x2_pool.tile([128, 4, D], F32, tag="x2")
            xpb_s = x2_pool.tile([128, 4, D], F32, tag="xpb")
            nc.sync.dma_start(xpb_s[:], xpb.rearrange("st p d -> p st d"))

            # Phase D: proj (own 512 tokens) + residual -> x2
            with tc.tile_pool(name="proj", bufs=1) as prj:
                pin = prj.tile([128, 16, 512], RDT, tag="pin")
                for hh in range(HPC):
                    for q in range(4):
                        for half in range(2):
                            src_row = 256 * (2 * q + half) + 64 * hh
                            nc.gpsimd.dma_start(
                                pin[64 * half:64 * (half + 1), 4 * hh + q, :],
                                bout_all[src_row:src_row + 64, :])
                wpd_s = prj.tile([128, 16, D], RDT, tag="wpd")
                nc.sync.dma_start(wpd_s[:], wpd.rearrange("et p d -> p et d"))
                for sc in range(4):
                    for dc in range(2):
                        acc = ps2.tile([128, 1024], F32, tag="ps2")
                        for et in range(16):
                            nc.tensor.matmul(
                                acc[:, 0:512],
                                pin[:, et, 128 * sc:128 * (sc + 1)],
                                wpd_s[:, et, 512 * dc:512 * (dc + 1)],
                                start=(et == 0), stop=(et == 15))
                        nc.vector.tensor_add(
                            x2[:, sc, 512 * dc:512 * (dc + 1)],
                            acc[:, 0:512],
                            xpb_s[:, sc, 512 * dc:512 * (dc + 1)])

            if phase == "D":
                nc.sync.dma_start(
                    dbg["x2"].rearrange("p (a b) -> p a b", a=4), x2[:])
                return

            # Phase E/F: LN2 -> h2T ; MLP ; output (LN2 g/b folded into W1/b1)
            with tc.tile_pool(name="mt", bufs=1) as mt_pool:
                mT = mt_pool.tile([128, 32, TOK], RDT, tag="mT")

                with tc.tile_pool(name="h2", bufs=1) as h2_pool:
                    h2T = h2_pool.tile([128, 8, TOK], RDT, tag="h2T")
                    with tc.tile_pool(name="ln2", bufs=2) as lnp2, \
                         tc.tile_pool(name="ln2s", bufs=4) as lns2:
                        for st in range(4):
                            _ln_transpose(nc, tc, ps2, lnp2, lns2,
                                          x2[:, st, :], h2T, st, idt, "b")
                    if phase == "E":
                        nc.sync.dma_start(
                            dbg["h2T"].rearrange("p (a b) -> p a b", a=8),
                            h2T[:])
                        return

                    # MLP up: mT = relu(W1'^T h2 + b1')
                    for fc in range(32):
                        wt = w1s.tile([128, 8, 128], RDT, tag="w1t")
                        nc.sync.dma_start(wt[:], w1[fc])
                        acc = ps2.tile([128, 1024], F32, tag="ps2")
                        for dt in range(8):
                            nc.tensor.matmul(
                                acc[:, 0:512], wt[:, dt, :], h2T[:, dt, :],
                                start=(dt == 0), stop=(dt == 7))
                        nc.scalar.activation(mT[:, fc, :], acc[:, 0:512],
                                             AF.Relu,
                                             bias=b1t_s[:, fc:fc + 1])

                # MLP down + residual + b2 -> y
                with tc.tile_pool(name="yout", bufs=2) as yp:
                    for dc in range(2):
                        accs = [ps2.tile([128, 1024], F32, tag="ps2",
                                         name=f"acy{dc}_{i}")
                                for i in range(2)]
                        for ft in range(32):
                            wt = w2s.tile([128, 512], RDT, tag="w2t")
                            nc.sync.dma_start(wt[:], w2[dc, ft])
                            for sc in range(4):
                                nc.tensor.matmul(
                                    accs[sc // 2][:, 512 * (sc % 2):
                                                  512 * (sc % 2 + 1)],
                                    mT[:, ft, 128 * sc:128 * (sc + 1)],
                                    wt[:],
                                    start=(ft == 0), stop=(ft == 31))
                        for sc in range(4):
                            yt = yp.tile([128, 512], F32, tag="yt")
                            nc.vector.tensor_add(
                                yt[:],
                                accs[sc // 2][:, 512 * (sc % 2):
                                              512 * (sc % 2 + 1)],
                                x2[:, sc, 512 * dc:512 * (dc + 1)])
                            nc.vector.tensor_add(
                                yt[:], yt[:],
                                b2b_s[:, 512 * dc:512 * (dc + 1)])
                            nc.sync.dma_start(
                                y[128 * sc:128 * (sc + 1),
                                  512 * dc:512 * (dc + 1)],
                                yt[:])


def _get_nc(phase="F"):
    key = f"nc_{phase}"
    if key not in _CACHE:
        _CACHE[key] = _build(phase)
    return _CACHE[key]


def _tf32_round(a):
    """Round fp32 to tf32 (10-bit mantissa) with round-to-nearest-even."""
    if RDT is F32:
        return np.ascontiguousarray(a)
    a = np.ascontiguousarray(a)
    b = a.view(np.uint32)
    lsb = (b >> np.uint32(13)) & np.uint32(1)
    out = (b + np.uint32(0xFFF) + lsb) & np.uint32(0xFFFFE000)
    return out.view(np.float32)


def _prep_in_maps(x, Wq, Wk, Wv, Wproj, bproj, ln1_g, ln1_b, ln2_g, ln2_b,
                  W1, b1, W2, b2):
    f = np.float32
    x = np.asarray(x, f)
    Wq = np.asarray(Wq, f); Wk = np.asarray(Wk, f); Wv = np.asarray(Wv, f)
    Wproj = np.asarray(Wproj, f); bproj = np.asarray(bproj, f)
    ln1_g = np.asarray(ln1_g, f); ln1_b = np.asarray(ln1_b, f)
    ln2_g = np.asarray(ln2_g, f); ln2_b = np.asarray(ln2_b, f)
    W1 = np.asarray(W1, f); b1 = np.asarray(b1, f)
    W2 = np.asarray(W2, f); b2 = np.asarray(b2, f)

    scale = f(HS) ** f(-0.5)
    ident = np.eye(128, dtype=f)
    onesv = np.ones((128, 128), dtype=f)
    # LN2 gain folded into W1 rows; LN2 bias folded into b1.
    W1g = W1 * ln2_g[:, None]
    b1f = b1 + ln2_b @ W1
    b1tv = np.ascontiguousarray(b1f.reshape(32, 128).T)
    b2bv = np.ascontiguousarray(np.tile(b2[None, :], (128, 1)))
    w1v = _tf32_round(W1g.reshape(8, 128, 32, 128).transpose(2, 1, 0, 3))
    w2v = _tf32_round(W2.reshape(32, 128, 2, 512).transpose(2, 0, 1, 3))

    in_maps = []
    for c in range(N_CORES):
        g, r = divmod(c, GROUP)
        heads = [4 * r + hh for hh in range(HPC)]
        # LN1 gain folded into QKV weight rows; LN1 bias -> bqk / bv.
        wq_c = np.concatenate([Wq[hh] for hh in heads], axis=1) * scale
        wk_c = np.concatenate([Wk[hh] for hh in heads], axis=1)
        wv_c = np.concatenate([Wv[hh] for hh in heads], axis=1)
        bq_c = ln1_b @ wq_c     # [256]
        bk_c = ln1_b @ wk_c
        bv_c = ln1_b @ wv_c
        wq_g = wq_c * ln1_g[:, None]
        wk_g = wk_c * ln1_g[:, None]
        wv_g = wv_c * ln1_g[:, None]
        wq_r = _tf32_round(wq_g.reshape(8, 128, 2, 128).transpose(1, 0, 2, 3))
        wk_r = _tf32_round(wk_g.reshape(8, 128, 2, 128).transpose(1, 0, 2, 3))
        wv_r = _tf32_round(wv_g.reshape(8, 128, 256).transpose(1, 0, 2))
        bqk_v = np.stack([bq_c.reshape(2, 128).T, bk_c.reshape(2, 128).T],
                         axis=2)  # [128, 2(pt), 2(q/k)]
        bv_v = _tf32_round(bv_c.reshape(1, 256))
        # zero-padded proj: e-tile (4*h_slot + i//2), rows 64*(i%2)
        wpd_v = np.zeros((16, 128, D), dtype=f)
        for h_slot in range(HPC):
            for i in range(N_CORES):
                if i // GROUP == g:
                    gh = 4 * (i % GROUP) + h_slot
                    blk = 64 * (i % 2)
                    wpd_v[4 * h_slot + i // 2, blk:blk + 64, :] = \
                        Wproj[64 * gh:64 * (gh + 1), :]
        wpd_v = _tf32_round(wpd_v)
        xrows = x[g, TOK * r:TOK * (r + 1), :] + bproj
        in_maps.append({
            "xb": np.ascontiguousarray(x[g]),
            "wq": wq_r, "wk": wk_r, "wv": wv_r,
            "bqk": np.ascontiguousarray(bqk_v), "bv": bv_v,
            "wpd": wpd_v, "w1": w1v, "w2": w2v,
            "xpb": np.ascontiguousarray(xrows.reshape(4, 128, D)),
            "b1t": b1tv, "b2b": b2bv, "ident": ident, "onesc": onesv,
        })
    return in_maps


def run(inputs, trace=False, phase="F"):
    nc = _get_nc(phase)
    in_maps = _prep_in_maps(**inputs)
    res = bass_utils.run_bass_kernel_spmd(
        nc, in_maps, core_ids=list(range(N_CORES)), trace=trace)
    if phase != "F":
        return res.results, res.exec_time_ns
    out = np.empty((B, S, D), dtype=np.float32)
    for c in range(N_CORES):
        g, r = divmod(c, GROUP)
        out[g, TOK * r:TOK * (r + 1), :] = res.results[c]["y"]
    return out, res.exec_time_ns


def kernel(**inputs):
    out, _ = run(inputs)
    return out


# revision 26
# speedup vs baseline: 2.5326x; 1.0902x over previous
"""Trainium2 Bass kernel for a dense pre-LN transformer block.

Sharding (8 NeuronCores):
  - 2 groups of 4 cores; group g handles batch g.
  - Within a group, attention is head-parallel: core owns 4 of 16 heads over
    the full 2048-token batch.
  - After attention, one 8-way AllToAll re-shards the attention output from
    head-parallel to token-parallel (each core ends with all 1024 o-features
    for its own 512 tokens).  Shards destined to the other batch group carry
    duplicate data and are neutralized by zero rows in a per-core zero-padded
    Wproj (the program is rank-invariant; only input data differs).
  - proj, LN2 and the MLP are token-parallel: core computes its own 512 rows.

Matmul inputs are float32r (tf32); accumulation, layernorm, softmax
normalization and the residual stream stay fp32.  The softmax needs no
max-subtraction (scores are O(1)); the denominator comes from a ones-column
appended to V inside the same PE accumulation.  LayerNorm gains/biases are
folded into the adjacent weights on the host.
"""

import os
import sys

if "/opt/trn_rl_repo" not in sys.path:
    sys.path.insert(0, "/opt/trn_rl_repo")

import numpy as np

import concourse.bass as bass
import concourse.mybir as mybir
from concourse import bacc, tile
from concourse import bass_utils

B, S, D, H = 2, 2048, 1024, 16
HS = D // H            # 64
EPS = 1e-5
N_CORES = 8
GROUP = 4              # cores per batch group
HPC = 4                # heads per core
TOK = S // GROUP       # own tokens per core (512)

F32 = mybir.dt.float32
F32R = mybir.dt.float32r
RDT = F32 if os.environ.get("KF32") else F32R
AF = mybir.ActivationFunctionType
ALU = mybir.AluOpType

_CACHE = {}

_PHASES = ["A", "B", "C0", "C", "D", "E", "F"]


def _build(phase="F"):
    nc = bacc.Bacc("TRN2", target_bir_lowering=False, debug=False,
                   enable_asserts=True, num_devices=N_CORES)

    tens = {}
    tens["xb"] = nc.dram_tensor("xb", [S, D], F32, kind="ExternalInput").ap()
    # QKV weights with LN1 gain folded; wq also carries the 1/sqrt(HS) scale.
    tens["wq"] = nc.dram_tensor("wq", [128, 8, 2, 128], RDT, kind="ExternalInput").ap()
    tens["wk"] = nc.dram_tensor("wk", [128, 8, 2, 128], RDT, kind="ExternalInput").ap()
    tens["wv"] = nc.dram_tensor("wv", [128, 8, 256], RDT, kind="ExternalInput").ap()
    tens["bqk"] = nc.dram_tensor("bqk", [128, 2, 2], F32, kind="ExternalInput").ap()
    tens["bv"] = nc.dram_tensor("bv", [1, 256], RDT, kind="ExternalInput").ap()
    tens["wpd"] = nc.dram_tensor("wpd", [16, 128, D], RDT, kind="ExternalInput").ap()
    tens["w1"] = nc.dram_tensor("w1", [32, 128, 8, 128], RDT, kind="ExternalInput").ap()
    tens["w2"] = nc.dram_tensor("w2", [2, 32, 128, 512], RDT, kind="ExternalInput").ap()
    tens["xpb"] = nc.dram_tensor("xpb", [4, 128, D], F32, kind="ExternalInput").ap()
    tens["b1t"] = nc.dram_tensor("b1t", [128, 32], F32, kind="ExternalInput").ap()
    tens["b2b"] = nc.dram_tensor("b2b", [128, D], F32, kind="ExternalInput").ap()
    tens["ident"] = nc.dram_tensor("ident", [128, 128], F32, kind="ExternalInput").ap()
    tens["onesc"] = nc.dram_tensor("onesc", [128, 128], RDT, kind="ExternalInput").ap()
    tens["y"] = nc.dram_tensor("y", [TOK, D], F32, kind="ExternalOutput").ap()

    dbg = {}
    if phase != "F":
        shapes = {
            "A": {"h1T": [128, 8 * S]},
            "B": {"QT": [128, 2 * S], "KT": [128, 2 * S], "Vt": [128, 16 * 4 * 65]},
            "C0": {"dbin_a": [1024, 512], "dbin_b": [1024, 512]},
            "C": {"dbout_a": [1024, 512], "dbout_b": [1024, 512]},
            "D": {"x2": [128, 4 * D]},
            "E": {"h2T": [128, 8 * TOK]},
        }[phase]
        for k, shp in shapes.items():
            dbg[k] = nc.dram_tensor(f"dbg_{k}", shp, F32,
                                    kind="ExternalOutput").ap()

    with tile.TileContext(nc) as tc:
        with nc.allow_low_precision(reason="tf32 matmul inputs by design"):
            _emit(nc, tc, tens, phase, dbg)
    nc.compile()
    return nc


def _ln_tile(nc, lns, src_ap, tag):
    """LayerNorm stats for one [128, 1024] tile -> (mean, rstd) [128,1].

    rstd gets one Newton refinement (ACT Sqrt has a loose ULP budget).
    """
    stats = lns.tile([128, 2, 6], F32, tag=f"{tag}stats")
    nc.vector.bn_stats(stats[:, 0, :], src_ap[:, 0:512])
    nc.vector.bn_stats(stats[:, 1, :], src_ap[:, 512:1024])
    mv = lns.tile([128, 2], F32, tag=f"{tag}mv")
    nc.vector.bn_aggr(mv[:], stats[:])
    ve = lns.tile([128, 1], F32, tag=f"{tag}ve")
    nc.vector.tensor_scalar_add(ve[:], mv[:, 1:2], EPS)
    std = lns.tile([128, 1], F32, tag=f"{tag}std")
    nc.scalar.activation(std[:], ve[:], AF.Sqrt)
    r0 = lns.tile([128, 1], F32, tag=f"{tag}r0")
    nc.vector.reciprocal(r0[:], std[:])
    t1 = lns.tile([128, 1], F32, tag=f"{tag}t1")
    nc.vector.tensor_mul(t1[:], r0[:], r0[:])
    nc.vector.tensor_mul(t1[:], t1[:], ve[:])
    nc.vector.tensor_scalar(t1[:], t1[:], -0.5, 1.5, ALU.mult, ALU.add)
    r1 = lns.tile([128, 1], F32, tag=f"{tag}r1")
    nc.vector.tensor_mul(r1[:], t1[:], r0[:])
    return mv[:, 0:1], r1


def _ln_transpose(nc, tc, ps2, lnp, lns, src_ap, dstT, st, idt, tag):
    """One [128,1024] tile: LN stats + prenorm + PE transpose into dstT."""
    mean, rstd = _ln_tile(nc, lns, src_ap, tag)
    pn = lnp.tile([128, D], F32, tag=f"{tag}pn")
    nc.vector.tensor_scalar(pn[:], src_ap, mean, rstd[:],
                            ALU.subtract, ALU.mult)
    pst = ps2.tile([128, 1024], F32, tag="ps2")
    for dt in range(8):
        nc.tensor.transpose(pst[:, 128 * dt:128 * (dt + 1)],
                            pn[:, 128 * dt:128 * (dt + 1)], idt[:])
    nc.vector.tensor_copy(
        dstT[:, :, 128 * st:128 * (st + 1)],
        pst[:].rearrange("p (dt t) -> p dt t", dt=8))


def _emit(nc, tc, tens, phase, dbg):
    xb, wpd, w1, w2, xpb, y = (tens["xb"], tens["wpd"], tens["w1"],
                               tens["w2"], tens["xpb"], tens["y"])

    with tc.tile_pool(name="const", bufs=1) as const, \
         tc.tile_pool(name="ps2", bufs=3, space="PSUM") as ps2, \
         tc.tile_pool(name="ps_av", bufs=2, space="PSUM") as ps_av, \
         tc.tile_pool(name="dram", bufs=1, space="DRAM") as dram, \
         tc.tile_pool(name="w1s", bufs=4) as w1s, \
         tc.tile_pool(name="w2s", bufs=4) as w2s:

        # ---------- constants; big weight tiles load after the first
        # ---------- x tiles (the emission point sets sync-queue order)
        idt = const.tile([128, 128], F32)
        nc.sync.dma_start(idt[:], tens["ident"][:])
        b1t_s = const.tile([128, 32], F32, tag="b1t")
        b2b_s = const.tile([128, D], F32, tag="b2b")
        onesc_s = const.tile([128, 128], RDT, tag="onesc")
        ones64 = onesc_s[0:1, 0:64]
        onestok = onesc_s[0:1, :]
        bqk_s = const.tile([128, 2, 2], F32, tag="bqk")
        bv_s = const.tile([1, 256], RDT, tag="bv")
        wq_s = const.tile([128, 8, 2, 128], RDT, tag="wq")
        wk_s = const.tile([128, 8, 2, 128], RDT, tag="wk")
        wv_s = const.tile([128, 8, 256], RDT, tag="wv")

        def _load_big_consts():
            nc.sync.dma_start(wq_s[:], tens["wq"][:])
            nc.sync.dma_start(wk_s[:], tens["wk"][:])
            nc.sync.dma_start(wv_s[:], tens["wv"][:])
            nc.sync.dma_start(bqk_s[:], tens["bqk"][:])
            nc.sync.dma_start(bv_s[:], tens["bv"][:])
            nc.sync.dma_start(onesc_s[:], tens["onesc"][:])
            nc.sync.dma_start(b1t_s[:], tens["b1t"][:])
            nc.sync.dma_start(b2b_s[:], tens["b2b"][:])

        # PE warm-up: ~7us of throwaway matmuls so HAM is at full clock
        # by the time the first transposes arrive.
        for wu in range(24):
            wps = ps2.tile([128, 1024], F32, tag="ps2")
            nc.tensor.matmul(wps[:, 0:128], idt[:], idt[:],
                             start=True, stop=True)

        bin_a = dram.tile([1024, 512], RDT, tag="bin_a", name="bin_a")
        bout_a = dram.tile([1024, 512], RDT, tag="bout_a", name="bout_a")
        bin_b = dram.tile([1024, 512], RDT, tag="bin_b", name="bin_b")
        bout_b = dram.tile([1024, 512], RDT, tag="bout_b", name="bout_b")

        # ====================== attention half ======================
        with tc.tile_pool(name="qkv", bufs=1) as qkv_pool:
            QT = qkv_pool.tile([128, 2, S], RDT, tag="QT")
            KT = qkv_pool.tile([128, 2, S], RDT, tag="KT")
            Vt = qkv_pool.tile([128, 16, 4, 65], RDT, tag="Vt")
            nc.sync.dma_start(
                Vt[:, :, :, 64],
                tens["onesc"][:, 0:64].rearrange("p (a b) -> p a b", a=16))

            # Phases A+B interleaved: LN1 -> h1T; QKV per 512-token chunk.
            # h1T: [128 (d in), 8 (d out), 2048]; QT/KT: [128, 2, 2048]
            # (partition tile pt = heads {2pt, 2pt+1});
            # V: [128, 16, 4, 65] ([t_in, t_out, head, e|ones])
            with tc.tile_pool(name="h1", bufs=1) as h1_pool:
                h1T = h1_pool.tile([128, 8, S], RDT, tag="h1T")
                with tc.tile_pool(name="ln1", bufs=3) as lnp, \
                     tc.tile_pool(name="ln1s", bufs=4) as lns:
                    for tc4 in range(4):
                        for st in range(4 * tc4, 4 * tc4 + 4):
                            xt = lnp.tile([128, D], F32, tag="xt")
                            nc.sync.dma_start(
                                xt[:], xb[128 * st:128 * (st + 1), :])
                            _ln_transpose(nc, tc, ps2, lnp, lns, xt[:],
                                          h1T, st, idt, "a")
                        if phase == "A" and tc4 == 3:
                            nc.sync.dma_start(
                                dbg["h1T"].rearrange("p (a b) -> p a b", a=8),
                                h1T[:])
                            return
                        # ---- QKV for this 512-token chunk ----
                        if tc4 == 0:
                            _load_big_consts()
                        tsl = slice(512 * tc4, 512 * (tc4 + 1))
                        for wten, dst, col in ((wq_s, QT, 0), (wk_s, KT, 1)):
                            for pt in range(2):
                                acc = ps2.tile([128, 1024], F32, tag="ps2")
                                for dt in range(8):
                                    nc.tensor.matmul(
                                        acc[:, 0:512], wten[:, dt, pt, :],
                                        h1T[:, dt, tsl],
                                        start=(dt == 0), stop=(dt == 7))
                                nc.vector.tensor_scalar_add(
                                    dst[:, pt, tsl], acc[:, 0:512],
                                    bqk_s[:, pt, col:col + 1])
                        for tc16 in range(4 * tc4, 4 * tc4 + 4):
                            acc = ps2.tile([128, 1024], F32, tag="ps2")
                            csl = slice(128 * tc16, 128 * (tc16 + 1))
                            for dt in range(8):
                                nc.tensor.matmul(
                                    acc[:, 0:256], h1T[:, dt, csl],
                                    wv_s[:, dt, :],
                                    start=(dt == 0), stop=False)
                            nc.tensor.matmul(acc[:, 0:256], onestok,
                                             bv_s[:], start=False, stop=True)
                            nc.vector.tensor_copy(
                                Vt[:, tc16, :, 0:64],
                                acc[:, 0:256].rearrange("p (h e) -> p h e",
                                                        h=4))

            if phase == "B":
                nc.sync.dma_start(
                    dbg["QT"].rearrange("p (a b) -> p a b", a=2), QT[:])
                nc.sync.dma_start(
                    dbg["KT"].rearrange("p (a b) -> p a b", a=2), KT[:])
                nc.sync.dma_start(
                    dbg["Vt"], Vt[:].rearrange("p a b c -> p (a b c)"))
                return

            # Phase C: scores + exp + AV per head pair
            with tc.tile_pool(name="et", bufs=6) as etp, \
                 tc.tile_pool(name="att", bufs=3) as att:
                for hp in range(2):
                    for sb in range(4):
                        pav = [ps_av.tile([65, 512], F32, tag="pav",
                                          name=f"pav{hp}_{sb}_{i}")
                               for i in range(2)]
                        for tcc in range(16):
                            psc = ps2.tile([128, 1024], F32, tag="ps2")
                            for hq in range(2):
                                nc.tensor.matmul(
                                    psc[:, 512 * hq:512 * (hq + 1)],
                                    KT[64 * hq:64 * (hq + 1), hp,
                                       128 * tcc:128 * (tcc + 1)],
                                    QT[64 * hq:64 * (hq + 1), hp,
                                       512 * sb:512 * (sb + 1)],
                                    start=True, stop=True)
                            et = etp.tile([128, 1024], RDT, tag="et")
                            nc.scalar.activation(et[:], psc[:], AF.Exp)
                            for hq in range(2):
                                nc.tensor.matmul(
                                    pav[hq][:],
                                    Vt[:, tcc, 2 * hp + hq, :],
                                    et[:, 512 * hq:512 * (hq + 1)],
                                    start=(tcc == 0), stop=(tcc == 15))
                        for hq in range(2):
                            h = 2 * hp + hq
                            rec = att.tile([1, 512], RDT, tag="rec")
                            nc.vector.reciprocal(rec[:], pav[hq][64:65, :])
                            pbct = ps2.tile([128, 1024], F32, tag="ps2")
                            pbc = pbct[0:64, 0:512]
                            nc.tensor.matmul(pbc, ones64, rec[:],
                                             start=True, stop=True)
                            rb = att.tile([64, 512], F32, tag="rb")
                            nc.vector.tensor_copy(rb[:], pbc)
                            ot = att.tile([64, 512], RDT, tag="ot")
                            nc.vector.tensor_mul(ot[:], pav[hq][0:64, :],
                                                 rb[:])
                            # both group slots (keeps wire data finite)
                            bin_hp = bin_a if hp == 0 else bin_b
                            nc.gpsimd.dma_start(
                                bin_hp[128 * sb + 64 * hq:
                                       128 * sb + 64 * (hq + 1), :], ot[:])
                            nc.gpsimd.dma_start(
                                bin_hp[128 * (sb + 4) + 64 * hq:
                                       128 * (sb + 4) + 64 * (hq + 1), :],
                                ot[:])
                    # fire this head-pair's A2A; hp=0's overlaps hp=1 compute
                    if sb == 3:
                        nc.gpsimd.collective_compute(
                            "AllToAll", ALU.bypass,
                            replica_groups=[list(range(N_CORES))],
                            ins=[(bin_a if hp == 0 else bin_b).opt()],
                            outs=[(bout_a if hp == 0 else bout_b).opt()])
            if phase == "C0":
                nc.sync.dma_start(dbg["dbin_a"], bin_a[:])
                nc.sync.dma_start(dbg["dbin_b"], bin_b[:])
                return
            if phase == "C":
                nc.sync.dma_start(dbg["dbout_a"], bout_a[:])
                nc.sync.dma_start(dbg["dbout_b"], bout_b[:])
                return

        # ====================== token-local half ======================
        with tc.tile_pool(name="x2p", bufs=1) as x2_pool:
            x2 = x2_pool.tile([128, 4, D], F32, tag="x2")
            xpb_s = x2_pool.tile([128, 4, D], F32, tag="xpb")
            nc.sync.dma_start(xpb_s[:], xpb.rearrange("st p d -> p st d"))

            # Phase D: proj (own 512 tokens) + residual -> x2
            with tc.tile_pool(name="proj", bufs=1) as prj:
                pin = prj.tile([128, 16, 512], RDT, tag="pin")
                for hh in range(HPC):
                    bout_hp = bout_a if hh < 2 else bout_b
                    hq = hh % 2
                    for q in range(4):
                        for half in range(2):
                            src_row = 128 * (2 * q + half) + 64 * hq
                            nc.gpsimd.dma_start(
                                pin[64 * half:64 * (half + 1), 4 * hh + q, :],
                                bout_hp[src_row:src_row + 64, :])
                wpd_s = prj.tile([128, 16, D], RDT, tag="wpd")
                nc.sync.dma_start(wpd_s[:], wpd.rearrange("et p d -> p et d"))
                for sc in range(4):
                    for dc in range(2):
                        acc = ps2.tile([128, 1024], F32, tag="ps2")
                        for et in range(16):
                            nc.tensor.matmul(
                                acc[:, 0:512],
                                pin[:, et, 128 * sc:128 * (sc + 1)],
                                wpd_s[:, et, 512 * dc:512 * (dc + 1)],
                                start=(et == 0), stop=(et == 15))
                        nc.vector.tensor_add(
                            x2[:, sc, 512 * dc:512 * (dc + 1)],
                            acc[:, 0:512],
                            xpb_s[:, sc, 512 * dc:512 * (dc + 1)])

            if phase == "D":
                nc.sync.dma_start(
                    dbg["x2"].rearrange("p (a b) -> p a b", a=4), x2[:])
                return

            # Phase E/F: LN2 -> h2T ; MLP ; output (LN2 g/b folded into W1/b1)
            with tc.tile_pool(name="mt", bufs=1) as mt_pool:
                mT = mt_pool.tile([128, 32, TOK], RDT, tag="mT")

                with tc.tile_pool(name="h2", bufs=1) as h2_pool:
                    h2T = h2_pool.tile([128, 8, TOK], RDT, tag="h2T")
                    with tc.tile_pool(name="ln2", bufs=2) as lnp2, \
                         tc.tile_pool(name="ln2s", bufs=4) as lns2:
                        for st in range(4):
                            _ln_transpose(nc, tc, ps2, lnp2, lns2,
                                          x2[:, st, :], h2T, st, idt, "b")
                    if phase == "E":
                        nc.sync.dma_start(
                            dbg["h2T"].rearrange("p (a b) -> p a b", a=8),
                            h2T[:])
                        return

                    # MLP up: mT = relu(W1'^T h2 + b1')
                    for fc in range(32):
                        wt = w1s.tile([128, 8, 128], RDT, tag="w1t")
                        nc.sync.dma_start(wt[:], w1[fc])
                        acc = ps2.tile([128, 1024], F32, tag="ps2")
                        for dt in range(8):
                            nc.tensor.matmul(
                                acc[:, 0:512], wt[:, dt, :], h2T[:, dt, :],
                                start=(dt == 0), stop=(dt == 7))
                        nc.scalar.activation(mT[:, fc, :], acc[:, 0:512],
                                             AF.Relu,
                                             bias=b1t_s[:, fc:fc + 1])

                # MLP down + residual + b2 -> y
                with tc.tile_pool(name="yout", bufs=2) as yp:
                    for dc in range(2):
                        accs = [ps2.tile([128, 1024], F32, tag="ps2",
                                         name=f"acy{dc}_{i}")
                                for i in range(2)]
                        for ft in range(32):
                            wt = w2s.tile([128, 512], RDT, tag="w2t")
                            nc.sync.dma_start(wt[:], w2[dc, ft])
                            for sc in range(4):
                                nc.tensor.matmul(
                                    accs[sc // 2][:, 512 * (sc % 2):
                                                  512 * (sc % 2 + 1)],
                                    mT[:, ft, 128 * sc:128 * (sc + 1)],
                                    wt[:],
                                    start=(ft == 0), stop=(ft == 31))
                        for sc in range(4):
                            yt = yp.tile([128, 512], F32, tag="yt")
                            nc.vector.tensor_add(
                                yt[:],
                                accs[sc // 2][:, 512 * (sc % 2):
                                              512 * (sc % 2 + 1)],
                                x2[:, sc, 512 * dc:512 * (dc + 1)])
                            nc.vector.tensor_add(
                                yt[:], yt[:],
                                b2b_s[:, 512 * dc:512 * (dc + 1)])
                            nc.sync.dma_start(
                                y[128 * sc:128 * (sc + 1),
                                  512 * dc:512 * (dc + 1)],
                                yt[:])


def _get_nc(phase="F"):
    key = f"nc_{phase}"
    if key not in _CACHE:
        _CACHE[key] = _build(phase)
    return _CACHE[key]


def _tf32_round(a):
    """Round fp32 to tf32 (10-bit mantissa) with round-to-nearest-even."""
    if RDT is F32:
        return np.ascontiguousarray(a)
    a = np.ascontiguousarray(a)
    b = a.view(np.uint32)
    lsb = (b >> np.uint32(13)) & np.uint32(1)
    out = (b + np.uint32(0xFFF) + lsb) & np.uint32(0xFFFFE000)
    return out.view(np.float32)


def _prep_in_maps(x, Wq, Wk, Wv, Wproj, bproj, ln1_g, ln1_b, ln2_g, ln2_b,
                  W1, b1, W2, b2):
    f = np.float32
    x = np.asarray(x, f)
    Wq = np.asarray(Wq, f); Wk = np.asarray(Wk, f); Wv = np.asarray(Wv, f)
    Wproj = np.asarray(Wproj, f); bproj = np.asarray(bproj, f)
    ln1_g = np.asarray(ln1_g, f); ln1_b = np.asarray(ln1_b, f)
    ln2_g = np.asarray(ln2_g, f); ln2_b = np.asarray(ln2_b, f)
    W1 = np.asarray(W1, f); b1 = np.asarray(b1, f)
    W2 = np.asarray(W2, f); b2 = np.asarray(b2, f)

    scale = f(HS) ** f(-0.5)
    ident = np.eye(128, dtype=f)
    onesv = np.ones((128, 128), dtype=f)
    # LN2 gain folded into W1 rows; LN2 bias folded into b1.
    W1g = W1 * ln2_g[:, None]
    b1f = b1 + ln2_b @ W1
    b1tv = np.ascontiguousarray(b1f.reshape(32, 128).T)
    b2bv = np.ascontiguousarray(np.tile(b2[None, :], (128, 1)))
    w1v = _tf32_round(W1g.reshape(8, 128, 32, 128).transpose(2, 1, 0, 3))
    w2v = _tf32_round(W2.reshape(32, 128, 2, 512).transpose(2, 0, 1, 3))

    in_maps = []
    for c in range(N_CORES):
        g, r = divmod(c, GROUP)
        heads = [4 * r + hh for hh in range(HPC)]
        # LN1 gain folded into QKV weight rows; LN1 bias -> bqk / bv.
        wq_c = np.concatenate([Wq[hh] for hh in heads], axis=1) * scale
        wk_c = np.concatenate([Wk[hh] for hh in heads], axis=1)
        wv_c = np.concatenate([Wv[hh] for hh in heads], axis=1)
        bq_c = ln1_b @ wq_c     # [256]
        bk_c = ln1_b @ wk_c
        bv_c = ln1_b @ wv_c
        wq_g = wq_c * ln1_g[:, None]
        wk_g = wk_c * ln1_g[:, None]
        wv_g = wv_c * ln1_g[:, None]
        wq_r = _tf32_round(wq_g.reshape(8, 128, 2, 128).transpose(1, 0, 2, 3))
        wk_r = _tf32_round(wk_g.reshape(8, 128, 2, 128).transpose(1, 0, 2, 3))
        wv_r = _tf32_round(wv_g.reshape(8, 128, 256).transpose(1, 0, 2))
        bqk_v = np.stack([bq_c.reshape(2, 128).T, bk_c.reshape(2, 128).T],
                         axis=2)  # [128, 2(pt), 2(q/k)]
        bv_v = _tf32_round(bv_c.reshape(1, 256))
        # zero-padded proj: e-tile (4*h_slot + i//2), rows 64*(i%2)
        wpd_v = np.zeros((16, 128, D), dtype=f)
        for h_slot in range(HPC):
            for i in range(N_CORES):
                if i // GROUP == g:
                    gh = 4 * (i % GROUP) + h_slot
                    blk = 64 * (i % 2)
                    wpd_v[4 * h_slot + i // 2, blk:blk + 64, :] = \
                        Wproj[64 * gh:64 * (gh + 1), :]
        wpd_v = _tf32_round(wpd_v)
        xrows = x[g, TOK * r:TOK * (r + 1), :] + bproj
        in_maps.append({
            "xb": np.ascontiguousarray(x[g]),
            "wq": wq_r, "wk": wk_r, "wv": wv_r,
            "bqk": np.ascontiguousarray(bqk_v), "bv": bv_v,
            "wpd": wpd_v, "w1": w1v, "w2": w2v,
            "xpb": np.ascontiguousarray(xrows.reshape(4, 128, D)),
            "b1t": b1tv, "b2b": b2bv, "ident": ident, "onesc": onesv,
        })
    return in_maps


def run(inputs, trace=False, phase="F"):
    nc = _get_nc(phase)
    in_maps = _prep_in_maps(**inputs)
    res = bass_utils.run_bass_kernel_spmd(
        nc, in_maps, core_ids=list(range(N_CORES)), trace=trace)
    if phase != "F":
        return res.results, res.exec_time_ns
    out = np.empty((B, S, D), dtype=np.float32)
    for c in range(N_CORES):
        g, r = divmod(c, GROUP)
        out[g, TOK * r:TOK * (r + 1), :] = res.results[c]["y"]
    return out, res.exec_time_ns


def kernel(**inputs):
    out, _ = run(inputs)
    return out
